# revision 2
# baseline (speedup 1.0000x reference)
"""Bass/Tile kernel for BilinearAttentionLayer on 8 NeuronCores.

out[b] = softmax(x[b] @ W @ x[b]^T / sqrt(D)) @ x[b]

Sharding: data-parallel over batch (8 batches -> 8 cores). Each core runs an
identical program on its own batch slice (x[b], W) -> out[b].

Per-core layout: the score matrix is kept transposed (scoresT[t, s]) so
every matmul operand is naturally oriented and no score-matrix transposes
are needed:
  xwT   = matmul(lhsT=W,   rhs=xT)      [e, s]
  prodT = matmul(lhsT=xT,  rhs=xwT)     [t, s]   (softmax axis = partitions)
  PT    = exp((prodT - rowmax)/sqrt(D))           (exact, safe softmax)
  out   = matmul(lhsT=PT,  rhs=x_nat)   [s, d]   (lands natural)
  rowsum rides the PV loop as N=1 matmuls against a ones column, landing
  directly in [s-partition, 1] layout for the per-partition normalization.
The only data transposes are 64 PE transposes of x itself.

Row max (softmax axis = partitions here): DVE max-accumulate across the 16
prodT PSUM tiles -> one GpSimd partition_all_reduce(max), whose output is
already replicated across all partitions -> DVE subtract on the staged raw
scores -> ScalarE Exp.  The per-row max makes the kernel robust to any
input values (the shifted exp never overflows and rowsum >= 1).

Dtypes: score path (xT, W, xwT) in float32r (1 cycle/row on the PE at
N=512); raw scores staged in f32; PT and x_nat in bf16 for the PV matmul.

Scheduling:
 * All input DMAs are issued up front into persistent staging buffers
   that alias the strip pool, split across BOTH hardware DGE queues.
   w_stage is allocated FIRST so that strips[0] (which is written while
   x_stage is still being consumed) aliases the early-dead w_stage slot
   instead of x_stage.
 * prod block 0 is interleaved tile-by-tile into the consume/xw stream:
   prod0 tile tt is emitted right after consume_tile(tt+4), so the PE
   starts N=512 matmuls ~10us into the run (as soon as xw_chunk(0) is
   ready) instead of after all 64 transposes.  This also keeps the PE
   p-state at full clock through the prologue (isolated matmuls after an
   idle gap run at half clock for the first ~3us).
 * Prologue PSUM->SBUF copies and the x_nat cast run on ScalarE (idle in
   the prologue; DVE carries the W copies, xwT copies and the max chain
   and was within ~10% of the prologue span).
 * Each PV block (stage_b) is zipped into the NEXT prod block's matmul
   stream at per-tile granularity; within each tile step the previous
   block's sub/exp is emitted BEFORE this tile's strip-copy/max, and the
   PV pops lag one tile.
 * The last block's sub/exp is front-loaded across the previous block's
   PV stream; the final block's normalize+store is split in halves across
   both DGE queues to shorten the drain tail.
 * Output DMAs go on the Sync queue (idle in steady state).
"""

import numpy as np

import concourse.mybir as mybir
import concourse.tile as tile
from concourse import bacc
from concourse import bass_isa
from concourse import bass_utils
from concourse.masks import make_identity

B = 8
S = 2048
D = 512
P = 128
SB = 512  # s-block width (one fp32 PSUM bank)

F32 = mybir.dt.float32
F32R = mybir.dt.float32r
BF16 = mybir.dt.bfloat16

SCALE = float(1.0 / np.sqrt(np.float64(D)))
AF = mybir.ActivationFunctionType


def build_nc(s=S, d=D):
    nd = d // P   # d/e tiles of 128
    nst = s // P  # s/t tiles of 128
    nsb = s // SB  # s-blocks
    nss = SB // P  # 128-chunks per s-block

    nc = bacc.Bacc(
        "TRN2",
        target_bir_lowering=False,
        debug=False,
        num_devices=B,
    )
    x_d = nc.dram_tensor("x", [s, d], F32, kind="ExternalInput").ap()
    w_d = nc.dram_tensor("w", [d, d], F32, kind="ExternalInput").ap()
    o_d = nc.dram_tensor("o", [s, d], F32, kind="ExternalOutput").ap()

    x_tiled = x_d.rearrange("(n p) d -> p n d", p=P)  # [128, nst, d]
    w_tiled = w_d.rearrange("(k p) e -> p k e", p=P)  # [128, nd, d]
    o_tiled = o_d.rearrange("(n p) d -> p n d", p=P)

    with tile.TileContext(nc) as tc:
        with (
            tc.tile_pool(name="const", bufs=1) as constp,
            tc.tile_pool(name="big", bufs=1) as bigp,
            tc.tile_pool(name="strip", bufs=2) as stripp,
            tc.tile_pool(name="ptp", bufs=2) as ptp,
            tc.tile_pool(name="bcast", bufs=2) as bcp,
            tc.tile_pool(name="outs", bufs=3) as outp,
            tc.tile_pool(name="acc", bufs=2) as accp,
            tc.tile_pool(name="small", bufs=2) as smallp,
            tc.tile_pool(name="mm", bufs=6, space="PSUM") as mmp,
            tc.tile_pool(name="tr", bufs=2, space="PSUM") as trp,
        ):
            ident = constp.tile([P, P], F32)
            make_identity(nc, ident[:])
            ones = constp.tile([P, 1], BF16)
            nc.vector.memset(ones[:], 1.0)
            x_nat = bigp.tile([P, nst, d], BF16)
            xT = bigp.tile([P, nd, s], F32R)
            w_sb = bigp.tile([P, nd, d], F32R)
            xwT = bigp.tile([P, nd, s], F32R)

            # ---- staging: x and W land in the strip pool's two buffers
            # (same shape+tag, dead before the aliasing strips are first
            # written).  w_stage FIRST: strips[0] then takes its slot and
            # only has to wait for the 4 early W copies, not for all of
            # x_stage's consumption.
            w_stage = stripp.tile([P, nst, SB], F32, tag="strip", name="wstg")
            x_stage = stripp.tile([P, nst, SB], F32, tag="strip", name="xstg")
            # x0/x1 first so the transposes can start ASAP; W right after
            # (needed by xw_chunk(0)); the rest of x streams behind.
            nc.sync.dma_start(x_stage[:, 0, :], x_tiled[:, 0, :])
            nc.scalar.dma_start(x_stage[:, 1, :], x_tiled[:, 1, :])
            nc.sync.dma_start(w_stage[:, 0:2, :], w_tiled[:, 0:2, :])
            nc.scalar.dma_start(w_stage[:, 2:4, :], w_tiled[:, 2:4, :])
            for st in range(2, nst):
                eng = nc.sync if st % 2 == 0 else nc.scalar
                eng.dma_start(x_stage[:, st, :], x_tiled[:, st, :])
            for kt in range(nd):
                nc.vector.tensor_copy(w_sb[:, kt, :], w_stage[:, kt, :])

            def consume_tile(st):
                # bf16 round for the PV matmul; on ScalarE (idle in the
                # prologue, while DVE carries the max chain + xw copies)
                nc.scalar.copy(x_nat[:, st, :], x_stage[:, st, :])
                # xT[p, dt, st*128+q] = x[st*128+q, dt*128+p]
                ps = trp.tile([P, nd, P], F32, tag="tr", name="trps")
                for dt in range(nd):
                    nc.tensor.transpose(
                        ps[:, dt, :],
                        x_stage[:, st, dt * P:(dt + 1) * P],
                        ident[:],
                    )
                nc.scalar.copy(xT[:, :, st * P:(st + 1) * P], ps[:])

            def xw_chunk(sb):
                # xwT[e, s-block] = sum_d W[d, e] x[s, d]
                for et in range(nd):
                    ps = mmp.tile([P, SB], F32, tag="mm", name="mmps")
                    for kt in range(nd):
                        nc.tensor.matmul(
                            ps[:],
                            w_sb[:, kt, et * P:(et + 1) * P],
                            xT[:, kt, sb * SB:(sb + 1) * SB],
                            start=(kt == 0),
                            stop=(kt == nd - 1),
                        )
                    nc.vector.tensor_copy(xwT[:, et, sb * SB:(sb + 1) * SB], ps[:])

            strips = [None] * nsb
            pts = [None] * nsb
            bcs = [None] * nsb
            accs = [None] * nsb

            def prod_tile(sb, tt):
                """One prodT tile of block sb: 4 matmuls + strip stage +
                bf16 max-accumulate."""
                ps = mmp.tile([P, SB], F32, tag="mm")
                for et in range(nd):
                    nc.tensor.matmul(
                        ps[:],
                        xT[:, et, tt * P:(tt + 1) * P],
                        xwT[:, et, sb * SB:(sb + 1) * SB],
                        start=(et == 0),
                        stop=(et == nd - 1),
                    )
                # stage raw scores (ScalarE) + max-accumulate (DVE).
                # The max chain is bf16: the shift cancels exactly in the
                # softmax ratio, only the overflow margin moves ~0.4%.
                nc.scalar.copy(strips[sb][:, tt, :], ps[:])
                acc_new = accp.tile([P, SB], BF16, tag="acc")
                if accs[sb] is None:
                    nc.vector.tensor_copy(acc_new[:], ps[:])
                else:
                    nc.vector.tensor_max(acc_new[:], ps[:], accs[sb][:])
                accs[sb] = acc_new

            def prod_reduce(sb):
                # row max, replicated across all partitions, on idle GpSimd
                bc = bcp.tile([P, SB], BF16, tag="bc", name="bc", bufs=1)
                nc.gpsimd.partition_all_reduce(
                    bc[:], accs[sb][:], channels=P,
                    reduce_op=bass_isa.ReduceOp.max,
                )
                bcs[sb] = bc

            def sub_exp(sb, tt):
                """shifted-exp of one staged tile (DVE sub + ScalarE exp)."""
                strip = strips[sb]
                nc.vector.tensor_sub(
                    strip[:, tt, :], strip[:, tt, :], bcs[sb][:]
                )
                nc.scalar.activation(
                    pts[sb][:, tt, :],
                    strip[:, tt, :],
                    AF.Exp,
                    scale=SCALE,
                )

            def stage_b_units(sb, split_tail=False):
                """The PV block as a list of small emission units, so it can
                be zipped into another block's prod stream at fine grain."""
                ptt = pts[sb]
                st = {}
                units = []

                def start_chunk(ss):
                    def f():
                        if "rs" not in st:
                            st["rs"] = trp.tile(
                                [P, nss], F32, tag="tr", name="rsps"
                            )
                        st[ss] = mmp.tile([P, d], F32, tag="mm", name="mmps")
                    return f

                def mm_unit(ss, tt):
                    def f():
                        # out[s, d] = sum_t P[s, t] x[t, d]; lhsT = PT.
                        # The rowsum rides as an N=1 matmul on the same lhsT
                        # (~25ns each): cheaper than any DVE/GpSimd variant.
                        nc.tensor.matmul(
                            st[ss][:],
                            ptt[:, tt, ss * P:(ss + 1) * P],
                            x_nat[:, tt, :],
                            start=(tt == 0),
                            stop=(tt == nst - 1),
                        )
                        nc.tensor.matmul(
                            st["rs"][:, ss:ss + 1],
                            ptt[:, tt, ss * P:(ss + 1) * P],
                            ones[:],
                            start=(tt == 0),
                            stop=(tt == nst - 1),
                        )
                    return f

                def norm_unit(ss):
                    def f():
                        rs_rec = smallp.tile(
                            [P, 1], F32, tag="rsrec", name="rsrec"
                        )
                        nc.vector.reciprocal(rs_rec[:], st["rs"][:, ss:ss + 1])
                        ot = outp.tile([P, d], F32, tag="ot", name="ot")
                        if split_tail:
                            # drain: halve the normalize + store and spread
                            # them over both DGE queues so the last bytes
                            # leave ~1.5us earlier.
                            h = d // 2
                            nc.vector.tensor_scalar_mul(
                                ot[:, :h], st[ss][:, :h], rs_rec[:])
                            nc.sync.dma_start(
                                o_tiled[:, sb * nss + ss, :h], ot[:, :h])
                            nc.vector.tensor_scalar_mul(
                                ot[:, h:], st[ss][:, h:], rs_rec[:])
                            nc.scalar.dma_start(
                                o_tiled[:, sb * nss + ss, h:], ot[:, h:])
                        else:
                            nc.vector.tensor_scalar_mul(
                                ot[:], st[ss][:], rs_rec[:])
                            # outputs go on the Sync queue: idle in steady
                            # state (Act carries the copies + exps).
                            nc.sync.dma_start(
                                o_tiled[:, sb * nss + ss, :], ot[:])
                    return f

                for ss in range(nss):
                    units.append(start_chunk(ss))
                    for tt in range(nst):
                        units.append(mm_unit(ss, tt))
                    units.append(norm_unit(ss))
                return units

            # ---- prologue: consume x tiles as they land; xw chunk every 4
            # tiles; prod block 0 interleaved tile-by-tile 4 tiles behind
            # (prod0 tile tt needs xT[tt] + xwT chunk 0).
            strips[0] = stripp.tile([P, nst, SB], F32, tag="strip", name="strip")
            pts[0] = ptp.tile([P, nst, SB], BF16, tag="pt", name="pt")
            for st in range(nst):
                consume_tile(st)
                if st % 4 == 3:
                    xw_chunk(st // 4)
                if st >= 4:
                    prod_tile(0, st - 4)
            for tt in range(nst - 4, nst):
                prod_tile(0, tt)
            prod_reduce(0)

            def run_prod(sb, prev, pvsb=None):
                """prodT tiles of block sb, zipped (per tile) with block
                `prev`'s sub/exp and block `pvsb`'s PV units."""
                strips[sb] = stripp.tile(
                    [P, nst, SB], F32, tag="strip", name="strip"
                )
                pts[sb] = ptp.tile([P, nst, SB], BF16, tag="pt", name="pt")
                units = stage_b_units(pvsb) if pvsb is not None else []
                ui = 0
                for tt in range(nst):
                    ps = mmp.tile([P, SB], F32, tag="mm")
                    for et in range(nd):
                        nc.tensor.matmul(
                            ps[:],
                            xT[:, et, tt * P:(tt + 1) * P],
                            xwT[:, et, sb * SB:(sb + 1) * SB],
                            start=(et == 0),
                            stop=(et == nd - 1),
                        )
                    # old-data work first on ScalarE/DVE (exp + sub of the
                    # previous block are ready to run; the strip copy and max
                    # of THIS tile gate on the matmuls that just issued)
                    if prev is not None:
                        sub_exp(prev, tt)
                    nc.scalar.copy(strips[sb][:, tt, :], ps[:])
                    acc_new = accp.tile([P, SB], BF16, tag="acc")
                    if accs[sb] is None:
                        nc.vector.tensor_copy(acc_new[:], ps[:])
                    else:
                        nc.vector.tensor_max(acc_new[:], ps[:], accs[sb][:])
                    accs[sb] = acc_new
                    # PV pops lag one tile so the first PV weight load never
                    # waits on the exp backlog.
                    target = tt * len(units) // (nst - 1)
                    while ui < target:
                        units[ui]()
                        ui += 1
                prod_reduce(sb)

            # software pipeline:
            #   [prologue+prod0] | prod(1)+exp(0) | prod(2)+exp(1)+PV(0) |
            #   prod(3)+exp(2)+PV(1) | PV(2)+exp(3) | PV(3)
            run_prod(1, 0)
            for sb in range(2, nsb):
                run_prod(sb, sb - 1, pvsb=sb - 2)
            units2 = stage_b_units(nsb - 2)
            ei = 0
            for i, u in enumerate(units2):
                u()
                # front-loaded: the last block's exps must clear ScalarE
                # before its PV starts right after this stream.
                target = min(nst, (i + 1) * nst * 5 // (3 * len(units2)))
                while ei < target:
                    sub_exp(nsb - 1, ei)
                    ei += 1
            while ei < nst:
                sub_exp(nsb - 1, ei)
                ei += 1
            for u in stage_b_units(nsb - 1, split_tail=True):
                u()

    nc.compile()
    return nc


_NC_CACHE = {}


def _get_nc():
    if "nc" not in _NC_CACHE:
        _NC_CACHE["nc"] = build_nc()
    return _NC_CACHE["nc"]


def kernel(x: np.ndarray, attn_matrix: np.ndarray) -> np.ndarray:
    assert x.shape == (B, S, D) and attn_matrix.shape == (D, D)
    nc = _get_nc()
    w = np.ascontiguousarray(attn_matrix, dtype=np.float32)
    in_maps = [
        {"x": np.ascontiguousarray(x[b], dtype=np.float32), "w": w}
        for b in range(B)
    ]
    res = bass_utils.run_bass_kernel_spmd(nc, in_maps, core_ids=list(range(B)))
    out = np.stack([res.results[b]["o"] for b in range(B)], axis=0)
    return out.astype(np.float32, copy=False)
